# revision 45
# baseline (speedup 1.0000x reference)
"""Trainium2 Bass kernel: batched attention  out = softmax(Q K^T) V  (no 1/sqrt(d) scale).

Shapes (hardcoded): Q, K, V: [4, 16, 2048, 128] fp32 -> out [4, 16, 2048, 128] fp32.

Sharding: B*H = 64 heads, data-parallel across 8 NeuronCores (8 heads per core).

Per-head device algorithm (transpose-free layout, S_T[k, q] per 128-key chunk):
  Host pre-transposes Q, K to [D, N] per head, rounds to fp16 (the 2^-11
  rounding perturbs softmax by ~1e-3 rel, inside the 2e-2 budget). V is fp16.

  The exp stream is split across TWO engines so the kernel runs at the PE
  (matmul) roofline instead of the ACT (exp) roofline. Every 128-key chunk
  of a 1024-wide q-half is one [128, 1024] PSUM tile (2 banks), S_T =
  k1c.T @ q1 (fp16 -> PSUM fp32), exp'd into a bf16 arena by:
    - ACT (2 of every 3 chunks, global stream so the max consecutive-ACT
      run is 2): nc.scalar.activation Exp, ~1.11us/chunk.
    - DVE (every 3rd chunk): Schraudolph bit-trick exp in ONE tensor_scalar
      op: i16 = rint(s*(128*log2 e) + (127*128 - 7.5)) written through an
      int16 bitcast of the bf16 arena == 2^(s*log2 e) with ~2% elementwise
      error that averages out in softmax (end-to-end 3.9e-3 vs the 2e-2
      gate; verified on HW: the DVE fp32->int16 convert rounds-to-nearest
      and both PSUM/SBUF sources work). ~1.22us/chunk at 1x from PSUM.
      Safe for s in (-88, +89); this problem's S range is [-68.4, +67.9].
  PSUM holds 3 rotating S tiles (6 banks) + the O accumulator (2 banks);
  the S-fill (PE) / exp (ACT|DVE) chain is triple-buffered with ~17%
  chain-latency headroom over the PE cadence (~0.87us/chunk: S 1024 cols +
  PV 1024 cols @2.4GHz). Per 16-chunk q-half: PE ~13.9us (the roofline),
  ACT ~11.9us, DVE ~13.2us (exps + ps_o drain + partial T tree).

  O_T accumulates in PSUM over 16 PV matmuls (vc.T @ E[c]) scheduled 4-6
  chunks behind the S stream, with the prev q-half's PV(12..15) + ps_o
  drain (DVE copy) spread over the first 3 chunk-slots of the next q-half
  so the single ps_o buffer turns around without stalling the PE. The
  normalizer partials (T = sum_c E[c]) are bf16 adds on DVE, emitted by a
  window scheduler that never places a tree op in front of a due exp in
  the DVE FIFO (pieces <= ~1.2us, emitted only into the slack window after
  a DVE exp, accounting for the ps_o copy, carrying leftovers forward).
  8 partial [128,1024] tiles ship per q-half across the sync+gpsimd DMA
  rings (the final q-half combines its A/X partials one level deeper to
  shrink the end-of-run DMA drain; host reads slots {0,1,4,6,7} there).
  The host finishes the T reduction (sum over tiles and partitions) and
  divides. 6 dummy warm-up matmuls run during the input-DMA fill so the
  PE HAM clock-gate opens (1.2 -> 2.4 GHz) before the real stream starts.

Measured (8 cores, same profiled conditions): 246.9-250.1us vs 295.8us for
the previous ACT-only-exp kernel (pair/single PSUM rotation), rel err
3.91e-3. Breakdown: ~220us PE-roofline stream + ~9us residual PE stalls +
~7us startup fill + ~13us tail (output drain + framework postamble).
Notes: GPSIMD tensor ops were probed and rejected (a concurrent gpsimd add
slows DVE ops ~4.7x via the shared POOL SBUF port); 1024-wide matmuls are
rejected by the ISA (s3d3_mm_num_elements caps output at 512 fp32).
"""

import sys

sys.path.insert(0, "/opt/trn_rl_repo")

import numpy as np

import concourse.bass as bass
import concourse.tile as tile
from concourse import bacc, mybir
from concourse.bass_utils import run_bass_kernel_spmd

B, H, N, D = 4, 16, 2048, 128
NCORES = 8
HPC = (B * H) // NCORES  # heads per core = 8
P = 128                  # partitions
NK = N // P              # key chunks per head = 16
QH = 2                   # q halves (1024 each) to fit PSUM
QHW = N // QH            # 1024
TW = 8 * QHW             # T partial-tile columns shipped per q-half
F32 = mybir.dt.float32
BF16 = mybir.dt.bfloat16
FP16 = mybir.dt.float16
I16 = mybir.dt.int16

# Schraudolph bf16 exp constants: i16 = rint(s*EXP_A + EXP_B); bf16(i16) ~ e^s.
# Valid for s in (-88, +89); this problem's S range is [-68.4, +67.9].
EXP_A = float(np.float32(128.0 * np.log2(np.e)))
EXP_B = float(np.float32(127.0 * 128.0 - 7.5))

CHUNK_NS = 870.0   # PE cadence per chunk (S 1024 + PV 1024 cols @2.4GHz + ldw)
DVE_EXP_NS = 1400.0  # DVE exp duration + sem slack


def build_nc():
    nc = bacc.Bacc(None, target_bir_lowering=False)

    q1_d = nc.dram_tensor("q1", [HPC, D, N], FP16, kind="ExternalInput")
    k1_d = nc.dram_tensor("k1", [HPC, D, N], FP16, kind="ExternalInput")
    v_d = nc.dram_tensor("v", [HPC, N, D], FP16, kind="ExternalInput")
    ot_d = nc.dram_tensor("ot", [HPC, D, N], BF16, kind="ExternalOutput")
    t_d = nc.dram_tensor("t", [HPC, QH, P, TW], BF16, kind="ExternalOutput")

    with tile.TileContext(nc) as tc:
        with (
            tc.tile_pool(name="io", bufs=2) as io_pool,
            tc.tile_pool(name="wrm", bufs=1) as wrm_pool,
            tc.tile_pool(name="arena", bufs=2) as arena_pool,
            tc.tile_pool(name="ts", bufs=2) as ts_pool,
            tc.tile_pool(name="tt", bufs=2) as tt_pool,
            tc.tile_pool(name="osb", bufs=2) as o_pool,
            tc.tile_pool(name="psT", bufs=3, space="PSUM") as psT_pool,
            tc.tile_pool(name="pso", bufs=1, space="PSUM") as pso_pool,
        ):
            # --- PE warm-up: ~8 dummy matmuls on zeros so the HAM clock
            # gate opens during the input-DMA fill, before the real stream.
            wt = wrm_pool.tile([P, 512], BF16, tag="w")
            nc.vector.memset(wt[:], 0.0)
            wpo = pso_pool.tile([P, QHW], F32, tag="o")
            for _ in range(6):
                nc.tensor.matmul(
                    wpo[:, 0:512], wt[:, 0:P], wt[:], start=True, stop=True
                )

            def load_head(h, cold=False):
                k1t = io_pool.tile([P, N], FP16, tag="k1")
                q1t = io_pool.tile([P, N], FP16, tag="q1")
                vt3 = io_pool.tile([P, NK, P], FP16, tag="vt")
                vre = v_d[h].rearrange("(c p) d -> p c d", p=P)
                if cold:
                    # critical path for exp(0): q1's second half leads the
                    # gpsimd ring (it gated the old startup), k1 chunk 0
                    # follows; k1 chunks 1-7 stream on sync behind q1a.
                    nc.gpsimd.dma_start(
                        out=q1t[:, 512:QHW], in_=q1_d[h][:, 512:QHW]
                    )
                    nc.sync.dma_start(out=q1t[:, 0:512], in_=q1_d[h][:, 0:512])
                    nc.gpsimd.dma_start(out=k1t[:, 0:P], in_=k1_d[h][:, 0:P])
                    nc.sync.dma_start(out=k1t[:, P:QHW], in_=k1_d[h][:, P:QHW])
                    nc.gpsimd.dma_start(out=k1t[:, QHW:N], in_=k1_d[h][:, QHW:N])
                    # vt[p, c, d] = V[h, c*128 + p, d]
                    nc.sync.dma_start(out=vt3[:, 0:8], in_=vre[:, 0:8])
                    nc.gpsimd.dma_start(out=vt3[:, 8:NK], in_=vre[:, 8:NK])
                    nc.sync.dma_start(out=q1t[:, QHW:N], in_=q1_d[h][:, QHW:N])
                else:
                    nc.sync.dma_start(out=k1t[:, 0:2 * P], in_=k1_d[h][:, 0:2 * P])
                    nc.sync.dma_start(out=q1t[:, 0:QHW], in_=q1_d[h][:, 0:QHW])
                    nc.sync.dma_start(out=vt3[:, 0:8], in_=vre[:, 0:8])
                    nc.sync.dma_start(
                        out=k1t[:, 2 * P:QHW], in_=k1_d[h][:, 2 * P:QHW]
                    )
                    nc.gpsimd.dma_start(out=vt3[:, 8:NK], in_=vre[:, 8:NK])
                    nc.sync.dma_start(out=k1t[:, QHW:N], in_=k1_d[h][:, QHW:N])
                    nc.sync.dma_start(out=q1t[:, QHW:N], in_=q1_d[h][:, QHW:N])
                return q1t, k1t, vt3.rearrange("p c d -> p (c d)")

            class Qh:
                def __init__(self, tiles, h, qh):
                    self.q1t, self.k1t, self.vt = tiles
                    self.h, self.qh = h, qh
                    self.q0 = qh * QHW
                    self.ps_o = pso_pool.tile([P, QHW], F32, tag="o")
                    self.arena = arena_pool.tile([P, NK * QHW], BF16, tag="e")
                    self.s = ts_pool.tile([P, 4096], BF16, tag="s")
                    self.ta1a = tt_pool.tile([P, 2048], BF16, tag="ta1a")
                    self.ta1b = tt_pool.tile([P, 2048], BF16, tag="ta1b")
                    self.tx2 = tt_pool.tile([P, QHW], BF16, tag="tx2")
                    self.ty1 = tt_pool.tile([P, QHW], BF16, tag="ty1")
                    self.ty2 = tt_pool.tile([P, QHW], BF16, tag="ty2")

                def S(self, c, pt):
                    for j in range(2):
                        nc.tensor.matmul(
                            pt[:, j * 512:(j + 1) * 512],
                            self.k1t[:, c * P:(c + 1) * P],
                            self.q1t[:, self.q0 + j * 512:
                                     self.q0 + (j + 1) * 512],
                            start=True,
                            stop=True,
                        )

                def exp_act(self, c, pt):
                    nc.scalar.activation(
                        self.arena[:, c * QHW:(c + 1) * QHW], pt[:],
                        mybir.ActivationFunctionType.Exp,
                    )

                def exp_dve(self, c, pt):
                    nc.vector.tensor_scalar(
                        self.arena[:, c * QHW:(c + 1) * QHW].bitcast(I16),
                        pt[:], EXP_A, EXP_B,
                        mybir.AluOpType.mult, mybir.AluOpType.add,
                    )

                def PV(self, c):
                    for j in range(2):
                        nc.tensor.matmul(
                            self.ps_o[:, j * 512:(j + 1) * 512],
                            self.vt[:, c * P:(c + 1) * P],
                            self.arena[:, c * QHW + j * 512:
                                       c * QHW + (j + 1) * 512],
                            start=(c == 0),
                            stop=(c == NK - 1),
                        )

                def cast_o(self):
                    o_sb = o_pool.tile([P, QHW], BF16, tag="osb")
                    nc.vector.tensor_copy(out=o_sb[:], in_=self.ps_o[:])
                    nc.sync.dma_start(
                        out=ot_d[self.h][:, self.q0:self.q0 + QHW], in_=o_sb[:]
                    )

                # --- T partial-tree fillers: (ready_local, cost_ns, fn).
                # deep=True (final q-half only): one more combine level so
                # the end-of-run DMA backlog is smaller; host then reads
                # slots [0,1,4,5] for this q-half.
                def fillers(self, deep=False):
                    a, s = self.arena, self.s
                    td = t_d[self.h, self.qh]

                    def a1a():  # (c0+c4, c1+c5)
                        nc.vector.tensor_add(
                            self.ta1a[:], a[:, 0:2048], a[:, 4096:6144]
                        )
                        if not deep:
                            nc.sync.dma_start(
                                out=td[:, 0:1024], in_=self.ta1a[:, 0:1024]
                            )
                            nc.gpsimd.dma_start(
                                out=td[:, 1024:2048], in_=self.ta1a[:, 1024:2048]
                            )

                    def a1b():  # (c2+c6, c3+c7)
                        nc.vector.tensor_add(
                            self.ta1b[:], a[:, 2048:4096], a[:, 6144:8192]
                        )
                        if not deep:
                            nc.sync.dma_start(
                                out=td[:, 2048:3072], in_=self.ta1b[:, 0:1024]
                            )
                            nc.gpsimd.dma_start(
                                out=td[:, 3072:4096], in_=self.ta1b[:, 1024:2048]
                            )

                    def a2():  # deep only: c0..7 pairs
                        nc.vector.tensor_add(
                            s[:, 2048:4096], self.ta1a[:], self.ta1b[:]
                        )
                        nc.sync.dma_start(out=td[:, 0:1024], in_=s[:, 2048:3072])
                        nc.gpsimd.dma_start(
                            out=td[:, 1024:2048], in_=s[:, 3072:4096]
                        )

                    def x1():  # (c8+c10, c9+c11)
                        nc.vector.tensor_add(
                            s[:, 0:2048], a[:, 8192:10240], a[:, 10240:12288]
                        )
                        if not deep:
                            nc.gpsimd.dma_start(
                                out=td[:, 4096:5120], in_=s[:, 0:1024]
                            )
                            nc.sync.dma_start(
                                out=td[:, 5120:6144], in_=s[:, 1024:2048]
                            )

                    def x2():  # deep only: c8..11
                        nc.vector.tensor_add(
                            self.tx2[:], s[:, 0:1024], s[:, 1024:2048]
                        )
                        nc.gpsimd.dma_start(out=td[:, 4096:5120], in_=self.tx2[:])

                    def y1():  # c12+c13
                        nc.vector.tensor_add(
                            self.ty1[:], a[:, 12288:13312], a[:, 13312:14336]
                        )
                        nc.sync.dma_start(out=td[:, 6144:7168], in_=self.ty1[:])

                    def y2():  # c14+c15
                        nc.vector.tensor_add(
                            self.ty2[:], a[:, 14336:15360], a[:, 15360:16384]
                        )
                        nc.gpsimd.dma_start(out=td[:, 7168:8192], in_=self.ty2[:])

                    out = [
                        (5, 1180, a1a),
                        (7, 1180, a1b),
                    ]
                    if deep:
                        out.append((7, 1180, a2))
                    out += [
                        (11, 1180, x1),
                    ]
                    if deep:
                        out.append((11, 650, x2))
                    out += [
                        (13, 650, y1),
                        (15, 650, y2),
                    ]
                    return out

            seq = [(h, qh) for h in range(HPC) for qh in range(QH)]
            tiles = load_head(0, cold=True)
            next_tiles = None
            prev = None
            # pending DVE tree ops: list of (ready_global, cost_ns, fn)
            pending = []
            n_chunks = len(seq) * NK

            for idx, (h, qh) in enumerate(seq):
                cur = Qh(tiles, h, qh)
                base = idx * NK
                last = idx == len(seq) - 1
                pending.extend(
                    (base + rl, cost, fn)
                    for rl, cost, fn in cur.fillers(deep=last)
                )
                for L in range(NK):
                    g = base + L
                    pst = psT_pool.tile([P, QHW], F32, tag="s")
                    cur.S(L, pst)
                    if g % 3 == 2:  # DVE-owned (global stream: max ACT run 2)
                        cur.exp_dve(L, pst)
                        # window until the next DVE exp (3 chunks of PE);
                        # spend the slack on ready tree ops, carry the rest.
                        budget = 3 * CHUNK_NS - DVE_EXP_NS
                        if L in (0, 1, 2):
                            budget -= 1250  # the DVE ps_o cast lands here
                        if g + 3 >= n_chunks:
                            budget = 1e9  # no more DVE exps: drain everything
                        while pending and pending[0][0] <= g and budget > 0:
                            _, cost, fn = pending.pop(0)
                            fn()
                            budget -= cost
                    else:
                        cur.exp_act(L, pst)
                    # PV schedule (lag 4): exp(c) gets ~3us before PV(c)
                    # needs the arena chunk; the DVE ps_o cast (emitted at
                    # L=3, right after e2 in the DVE FIFO) completes before
                    # PV(0) at L=5 of the next q-half.
                    if prev is not None:
                        if L == 0:
                            prev.PV(12)
                        elif L == 1:
                            prev.PV(13)
                            prev.PV(14)
                        elif L == 2:
                            prev.PV(15)
                            prev.cast_o()
                            prev = None
                    if L == 5:
                        cur.PV(0)
                        cur.PV(1)
                    elif L >= 6:
                        cur.PV(L - 4)
                    if L == 4 and qh == 0 and h + 1 < HPC:
                        next_tiles = load_head(h + 1)
                if qh == 1:
                    tiles = next_tiles
                prev = cur

            prev.PV(12)
            prev.PV(13)
            prev.PV(14)
            while pending:  # the last q-half's remaining tree ops
                _, _, fn = pending.pop(0)
                fn()
            prev.PV(15)
            prev.cast_o()
    nc.finalize()
    return nc


def _f16_t(x):
    """[heads, N, D] fp32 -> transposed [heads, D, N] fp16."""
    return np.ascontiguousarray(x.transpose(0, 2, 1)).astype(np.float16)


def _prepare_in_maps(Q, K, V):
    Qf = np.asarray(Q, dtype=np.float32).reshape(B * H, N, D)
    Kf = np.asarray(K, dtype=np.float32).reshape(B * H, N, D)
    Vf = np.asarray(V, dtype=np.float32).reshape(B * H, N, D).astype(np.float16)
    q1 = _f16_t(Qf)
    k1 = _f16_t(Kf)
    in_maps = []
    for i in range(NCORES):
        s = slice(i * HPC, (i + 1) * HPC)
        in_maps.append({"q1": q1[s], "k1": k1[s], "v": Vf[s]})
    return in_maps


def run(Q, K, V, trace=False, **kwargs):
    nc = build_nc()
    in_maps = _prepare_in_maps(Q, K, V)
    res = run_bass_kernel_spmd(nc, in_maps, list(range(NCORES)), trace=trace, **kwargs)
    OT = np.concatenate([res.results[i]["ot"] for i in range(NCORES)], axis=0)
    T = np.concatenate([res.results[i]["t"] for i in range(NCORES)], axis=0)
    # l[head, qh, q] = sum over the partial tiles and 128 partitions.
    # Each core's final q-half (head HPC-1, qh=1) shipped deeper A/X
    # partials: only slots {0,1,4,6,7} are valid there.
    Tf = T.astype(np.float32).reshape(B * H, QH, P, 8, QHW)
    l = Tf.sum(axis=(2, 3))
    deep = Tf[HPC - 1::HPC, 1].sum(axis=1)  # [NCORES, 8, QHW]
    l[HPC - 1::HPC, 1] = deep[:, [0, 1, 4, 6, 7]].sum(axis=1)
    l = l.reshape(B * H, N)
    out = OT.astype(np.float32) / l[:, None, :]
    out = out.transpose(0, 2, 1).reshape(B, H, N, D)
    return np.ascontiguousarray(out), res


def kernel(Q, K, V):
    out, _ = run(Q, K, V, trace=False)
    return out
